# revision 1
# baseline (speedup 1.0000x reference)
"""GQA causal attention block (RoPE, 32 Q heads / 8 KV heads, S=2048, D=4096)
distributed tensor-parallel over heads across 8 TRN2 NeuronCores.

Per core c:
  - 4 query heads (wq cols 512c:512c+512), 1 KV head (wk/wv cols 128c:128c+128)
  - projections computed transposed (qT/kT/vT [hd, seq]) from host-transposed
    xT, weights as stationary operands, bf16 matmuls with f32 PSUM accum
  - RoPE applied with host tables; the half-rotation uses a PE permutation
    matmul (cross-partition moves are impossible on DVE)
  - attention computed as S^T [j, i] chunks -> exp (ACT, scale folded in) ->
    P^T tiles feed the PV matmul directly (no per-tile PE transposes);
    row sums via a ones-vector matmul; causal handled by skipping future
    j-tiles + additive -1e9 masks on the 4 diagonal tile variants
  - attention output O^T [hd, seq] normalized via a broadcast DMA of 1/l
    (DRAM round-trip), written bf16 to an AllGather bounce; 4 chunked
    AllGathers (one per 512-seq group) overlap comm with compute
  - output projection computed transposed (outT = wo_c^T @ Y^T) so the
    stationary operands come from SBUF-resident wo and the moving operands
    are [128,512] gathered-Y tiles; host transposes each shard back
Host gathers by concatenating the 8 (transposed) column shards.
"""

import numpy as np
import ml_dtypes

import concourse.bass as bass
import concourse.mybir as mybir
import concourse.tile as tile
from concourse.bass_utils import run_bass_kernel_spmd

bf16 = mybir.dt.bfloat16
f32 = mybir.dt.float32

NCORES = 8
S = 2048
DIM = 4096
HD = 128
NH = 32
QH = NH // NCORES          # 4 query heads per core
QW = QH * HD               # 512 wq cols per core
ROPE_BASE = 10000.0
SCALE = float(HD) ** -0.5
NEG = -1.0e9
NSEQ = S // 512            # 4 seq chunks
KT = DIM // 128            # 32 contraction tiles

_CACHE = {}


def _legalize_waits(nc, allowed_default=1):
    """This walrus build rejects instructions carrying more inline sync waits
    than the opcode template allows (0 for Drain, 1 elsewhere). Spill excess
    waits onto standalone EventSemaphore instructions inserted immediately
    before, on the same engine (engine order preserves semantics)."""
    for f in nc.m.functions:
        for bb in f.blocks:
            out = []
            for ins in bb.instructions:
                tname = type(ins).__name__
                si = getattr(ins, "sync_info", None)
                waits = list(si.on_wait) if (si is not None and si.on_wait) else []
                if tname == "InstEventSemaphore":
                    allowed = len(waits)
                elif tname == "InstDrain":
                    allowed = 0
                else:
                    allowed = allowed_default
                if len(waits) > allowed:
                    spill, keep = waits[allowed:], waits[:allowed]
                    for i, w in enumerate(spill):
                        ev = mybir.InstEventSemaphore(
                            name=f"{ins.name}_wfix{i}",
                            engine=ins.engine, ins=[], outs=[],
                        )
                        ev.sync_info = mybir.SyncInfo(on_wait=[w], on_update=[])
                        out.append(ev)
                    si.on_wait = keep
                out.append(ins)
            bb.instructions[:] = out


def _build_nc():
    nc = bass.Bass(num_devices=NCORES)

    xT = nc.declare_dram_parameter("xT", [DIM, S], bf16, isOutput=False)
    wq = nc.declare_dram_parameter("wq", [DIM, QW], bf16, isOutput=False)
    wk = nc.declare_dram_parameter("wk", [DIM, HD], bf16, isOutput=False)
    wv = nc.declare_dram_parameter("wv", [DIM, HD], bf16, isOutput=False)
    wo = nc.declare_dram_parameter("wo", [DIM, QW], bf16, isOutput=False)
    cosT = nc.declare_dram_parameter("cosT", [HD, S], f32, isOutput=False)
    sinT = nc.declare_dram_parameter("sinT", [HD, S], f32, isOutput=False)
    maskT = nc.declare_dram_parameter("maskT", [4, 128, 512], bf16, isOutput=False)
    perm = nc.declare_dram_parameter("perm", [128, 128], bf16, isOutput=False)
    ident = nc.declare_dram_parameter("ident", [128, 128], bf16, isOutput=False)
    outT = nc.declare_dram_parameter("outT", [QW, S], f32, isOutput=True)

    ag_in = nc.dram_tensor("ag_in", [4, QW, 512], bf16)
    ag_outs = [
        nc.dram_tensor(f"ag_out{g}", [NCORES, QW, 512], bf16, addr_space="Shared")
        for g in range(4)
    ]
    linv_dram = nc.dram_tensor("linv_dram", [4, QH, 512], f32)

    with tile.TileContext(nc) as tc:
        with (
            tc.tile_pool(name="const", bufs=1) as constp,
            tc.tile_pool(name="acts", bufs=1) as acts,
            tc.tile_pool(name="xin", bufs=6) as xin,
            tc.tile_pool(name="rope", bufs=2) as rope,
            tc.tile_pool(name="pt", bufs=6) as ptp,
            tc.tile_pool(name="epi", bufs=3) as epi,
            tc.tile_pool(name="cproj", bufs=6) as cproj,
            tc.tile_pool(name="psum", bufs=1, space="PSUM") as psum,
        ):
            def pbank(b, shape=(128, 512), dtype=f32, name="ps"):
                return psum.tile(list(shape), dtype, tag=f"b{b}",
                                 name=f"{name}_b{b}", bufs=1)

            # small constants first (cheap DMAs)
            perm_sb = constp.tile([128, 128], bf16)
            nc.sync.dma_start(perm_sb[:], perm[:])
            ident_sb = constp.tile([128, 128], bf16)
            nc.sync.dma_start(ident_sb[:], ident[:])
            ones_sb = constp.tile([128, 1], bf16)
            nc.vector.memset(ones_sb[:], 1.0)

            # weights: declared now, DMA'd lazily below to keep the
            # preamble off the first matmul's critical path
            wq_sb = constp.tile([128, KT, QW], bf16)
            wk_sb = constp.tile([128, KT, HD], bf16)
            wv_sb = constp.tile([128, KT, HD], bf16)
            wo_sb = constp.tile([128, KT, QW], bf16)
            cos_sb = constp.tile([HD, S], f32)
            sin_sb = constp.tile([HD, S], f32)
            mask_sb = constp.tile([128, 4, 512], bf16)

            # activations that live through attention
            qTr = acts.tile([128, QH, S], bf16)      # 4 head tiles [hd, seq]
            kTr = acts.tile([128, S], bf16)
            v_sb = acts.tile([128, S], bf16)         # 16 [seq,hd] tiles at jt*128

            wqr = wq.rearrange("(a p) m -> p a m", p=128)
            wkr = wk.rearrange("(a p) m -> p a m", p=128)
            wvr = wv.rearrange("(a p) m -> p a m", p=128)

            # ---- phase A: projections + rope ----
            for n in range(NSEQ):
                sl = bass.ts(n, 512)
                q_ps = [pbank(m, name="q") for m in range(QH)]
                k_ps = pbank(4, name="k")
                vT_ps = pbank(5, name="vT")
                for k in range(KT):
                    if n == 0:
                        nc.sync.dma_start(wq_sb[:, k], wqr[:, k])
                        nc.sync.dma_start(wk_sb[:, k], wkr[:, k])
                        nc.sync.dma_start(wv_sb[:, k], wvr[:, k])
                    x_sb = xin.tile([128, 512], bf16, tag="x")
                    nc.sync.dma_start(x_sb[:], xT[bass.ts(k, 128), sl])
                    st, sp = (k == 0), (k == KT - 1)
                    for m in range(QH):
                        nc.tensor.matmul(q_ps[m][:], wq_sb[:, k, bass.ts(m, 128)],
                                         x_sb[:], start=st, stop=sp)
                    nc.tensor.matmul(k_ps[:], wk_sb[:, k], x_sb[:], start=st, stop=sp)
                    nc.tensor.matmul(vT_ps[:], wv_sb[:, k], x_sb[:], start=st, stop=sp)

                if n == 0:
                    nc.sync.dma_start(cos_sb[:], cosT[:])
                    nc.sync.dma_start(sin_sb[:], sinT[:])
                    nc.sync.dma_start(mask_sb[:], maskT.rearrange("a p m -> p a m"))

                # rope: first free the accumulation banks (copy + cos-mul),
                # then the sw-products and adds
                t_bfs, t1s, sw_pss = [], [], []
                for idx in range(QH + 1):
                    src = q_ps[idx] if idx < QH else k_ps
                    t_bf = rope.tile([128, 512], bf16, tag=f"tbf{idx}",
                                     name=f"tbf{idx}", bufs=1)
                    nc.scalar.copy(t_bf[:], src[:])
                    t1 = rope.tile([128, 512], f32, tag=f"t1_{idx}",
                                   name=f"t1_{idx}", bufs=1)
                    nc.vector.tensor_mul(t1[:], src[:], cos_sb[:, sl])
                    t_bfs.append(t_bf)
                    t1s.append(t1)
                for idx in range(QH + 1):
                    dst = qTr[:, idx, sl] if idx < QH else kTr[:, sl]
                    sw_ps = pbank(6 + (idx % 2), name="sw")
                    nc.tensor.matmul(sw_ps[:], perm_sb[:], t_bfs[idx][:],
                                     start=True, stop=True)
                    t2 = rope.tile([128, 512], f32, tag=f"t2_{idx % 2}",
                                   name=f"t2_{idx % 2}")
                    nc.vector.tensor_mul(t2[:], sw_ps[:], sin_sb[:, sl])
                    nc.vector.tensor_add(dst, t1s[idx][:], t2[:])

                # v: copy vT chunk, transpose 128-blocks into [seq, hd] tiles
                v_bf = rope.tile([128, 512], bf16, tag="vbf")
                nc.scalar.copy(v_bf[:], vT_ps[:])
                for t in range(4):
                    vt_ps = pbank(6 + (t % 2), shape=(128, 128), dtype=bf16,
                                  name="vt")
                    nc.tensor.transpose(vt_ps[:], v_bf[:, bass.ts(t, 128)],
                                        ident_sb[:])
                    nc.any.tensor_copy(out=v_sb[:, bass.ts(4 * n + t, 128)],
                                       in_=vt_ps[:])

            # wo loads queued after phase A traffic, before phase C needs them
            wor = wo.rearrange("(a p) m -> p a m", p=128)
            for k in range(KT):
                nc.sync.dma_start(wo_sb[:, k], wor[:, k])

            # ---- phase B: attention, S^T layout, chunked AllGather ----
            for g in range(4):
                isl = bass.ts(g, 512)
                njt = 4 * g + 4
                for h in range(QH):
                    oT_ps = pbank(3 if h % 2 == 0 else 4, name="oT")
                    l_ps = pbank(5 if h % 2 == 0 else 6, shape=(1, 512), name="l")
                    STB = (0, 1, 2, 7)
                    for jt in range(njt):
                        st_ps = pbank(STB[jt % 4], name="st")
                        r = jt - 4 * g
                        if r >= 0:
                            nc.tensor.matmul(st_ps[:], ident_sb[:], mask_sb[:, r],
                                             start=True, stop=False)
                        nc.tensor.matmul(st_ps[:], kTr[:, bass.ts(jt, 128)],
                                         qTr[:, h, isl], start=(r < 0), stop=True)
                        pt = ptp.tile([128, 512], bf16, tag="pt")
                        nc.scalar.activation(pt[:], st_ps[:],
                                             mybir.ActivationFunctionType.Exp,
                                             scale=SCALE)
                        nc.tensor.matmul(l_ps[:], ones_sb[:], pt[:],
                                         start=(jt == 0), stop=(jt == njt - 1))
                        nc.tensor.matmul(oT_ps[:], v_sb[:, bass.ts(jt, 128)], pt[:],
                                         start=(jt == 0), stop=(jt == njt - 1))
                    linv = epi.tile([1, 512], f32, tag="linv")
                    nc.vector.reciprocal(linv[:], l_ps[:])
                    nc.sync.dma_start(linv_dram[g, h], linv[:])
                    lb = epi.tile([128, 512], f32, tag="lb")
                    nc.sync.dma_start(
                        lb[:], linv_dram[g, bass.ds(h, 1)].broadcast_to((128, 512)))
                    oT_sb = epi.tile([128, 512], bf16, tag="otsb")
                    nc.vector.tensor_mul(oT_sb[:], oT_ps[:], lb[:])
                    nc.sync.dma_start(ag_in[g, bass.ts(h, 128)], oT_sb[:])
                nc.gpsimd.collective_compute(
                    "AllGather", mybir.AluOpType.bypass,
                    replica_groups=[list(range(NCORES))],
                    ins=[ag_in[g]], outs=[ag_outs[g][:]],
                )

            # ---- phase C: outT = wo_c^T @ Y^T, wo stationary from SBUF ----
            for ns in range(NSEQ):
                o_ps = [pbank((0 if ns % 2 == 0 else 4) + ob, name="o")
                        for ob in range(QH)]
                for kt in range(KT):
                    c, db = kt // 4, kt % 4
                    y_sb = cproj.tile([128, 512], bf16, tag="y")
                    nc.sync.dma_start(y_sb[:], ag_outs[ns][c, bass.ts(db, 128)])
                    for ob in range(QH):
                        nc.tensor.matmul(
                            o_ps[ob][:], wo_sb[:, kt, bass.ts(ob, 128)], y_sb[:],
                            start=(kt == 0), stop=(kt == KT - 1))
                for ob in range(QH):
                    o_sb = cproj.tile([128, 512], f32, tag="osb")
                    nc.scalar.copy(o_sb[:], o_ps[ob][:])
                    nc.sync.dma_start(outT[bass.ts(ob, 128), bass.ts(ns, 512)],
                                      o_sb[:])

    _legalize_waits(nc)
    return nc


def _host_inputs(x, wq, wk, wv, wo):
    x = np.asarray(x, dtype=np.float32)
    xT = np.ascontiguousarray(x.reshape(S, DIM).T).astype(ml_dtypes.bfloat16)

    # rope tables in [hd, seq] layout with the sign of sin baked in
    inv_freq = 1.0 / ROPE_BASE ** (np.arange(0, HD, 2, dtype=np.float32) / HD)
    t = np.arange(S, dtype=np.float32)
    freqs = np.outer(inv_freq, t)                       # [64, S]
    cosT = np.concatenate([np.cos(freqs), np.cos(freqs)], 0).astype(np.float32)
    sinT = np.concatenate([-np.sin(freqs), np.sin(freqs)], 0).astype(np.float32)

    # S^T-layout diagonal masks: maskT[r][j, i] = 0 if r*128 + j <= i else NEG
    j = np.arange(128)[None, :, None]
    i = np.arange(512)[None, None, :]
    r = np.arange(4)[:, None, None]
    maskT = np.where(r * 128 + j <= i, 0.0, NEG).astype(ml_dtypes.bfloat16)

    perm = np.zeros((128, 128), dtype=np.float32)
    perm[np.arange(128), (np.arange(128) + 64) % 128] = 1.0
    ident = np.eye(128, dtype=np.float32)

    shared = {
        "xT": xT,
        "cosT": cosT,
        "sinT": sinT,
        "maskT": maskT,
        "perm": perm.astype(ml_dtypes.bfloat16),
        "ident": ident.astype(ml_dtypes.bfloat16),
    }
    maps = []
    for c in range(NCORES):
        m = dict(shared)
        m["wq"] = np.asarray(wq[:, c * QW:(c + 1) * QW]).astype(ml_dtypes.bfloat16)
        m["wk"] = np.asarray(wk[:, c * HD:(c + 1) * HD]).astype(ml_dtypes.bfloat16)
        m["wv"] = np.asarray(wv[:, c * HD:(c + 1) * HD]).astype(ml_dtypes.bfloat16)
        m["wo"] = np.asarray(wo[:, c * QW:(c + 1) * QW]).astype(ml_dtypes.bfloat16)
        maps.append(m)
    return maps


LAST_RESULT = {}


def kernel(x, wq, wk, wv, wo, mask=None, trace=False):
    if "nc" not in _CACHE:
        _CACHE["nc"] = _build_nc()
    nc = _CACHE["nc"]
    in_maps = _host_inputs(x, wq, wk, wv, wo)
    res = run_bass_kernel_spmd(nc, in_maps, list(range(NCORES)), trace=trace)
    LAST_RESULT["exec_time_ns"] = res.exec_time_ns
    LAST_RESULT["profile_json"] = res.profile_json
    it = res.instructions_and_trace
    LAST_RESULT["trace_dir"] = it if isinstance(it, str) else None
    full = np.concatenate(
        [res.results[c]["outT"].T for c in range(NCORES)], axis=1)
    return np.ascontiguousarray(full).reshape(1, S, DIM).astype(np.float32)

